# revision 1
# baseline (speedup 1.0000x reference)
"""Trainium2 Bass kernel for ConvexLinearAttention (elu(x)+1 linear attention).

Full-input contract: kernel(**inputs) takes the unsharded tensors
(x [2,2048,1024], wq/wk/wv/wo [1024,1024], bq/bk/bv/bo [1024]) and returns the
full output [2,2048,1024].

Sharding (8 cores): data-parallel over batch (2) x head-group-parallel (4 groups
of 4 heads).  Each core projects only its 256-wide head slice, runs the
linearized attention (attended = qf @ (kf^T V) / (qf @ sum(kf)) -- an exact
refactoring of the dense normalized scores), and emits a partial output
projection.  The host sums the 4 head-group partials per batch (the
tensor-parallel unshard).

All matmuls are float32r with moving dim 512 (full-rate fp32 mode); K and V
projections share one matmul via host-concatenated [wk|wv] weights so every
fp32 LDWEIGHTS hides under an N=512 matmul.  Q-projection is interleaved into
the K/V phase per s-chunk so the PE stays busy (HAM warm) while DMA streams x.
There are no on-chip transposes: operand orientations come from host-side
transposed layouts, and the per-head KV contraction uses block-diagonal-masked
KV/ksum tiles so cross-head terms vanish.
"""

from contextlib import ExitStack

import numpy as np

import concourse.bass as bass
import concourse.mybir as mybir
import concourse.tile as tile
from concourse import bacc, bass_utils

F32 = mybir.dt.float32
F32R = mybir.dt.float32r
AF = mybir.ActivationFunctionType
ALU = mybir.AluOpType

D = 1024          # model dim
S = 2048          # sequence length
BATCH = 2
CSL = 256         # head-slice width per core (4 heads x 64)
NG = 2            # 128-wide c-groups per core
P = 128
NDT = D // P      # 8 d-tiles
NST = S // P      # 16 s-tiles
SC = 512          # s-chunk (matmul moving dim)
NSC = S // SC     # 4 s-chunks
STC = SC // P     # 4 s-tiles per chunk
NET = D // P      # 8 e-tiles
EPS = 1e-6

_CACHE: dict = {}


def install_ntff_hook_shim():
    """Provide ``antenv.axon_hooks`` when the image ships only the antenv stub.

    concourse.bass_utils imports it unconditionally on the axon trace path;
    without this shim trace=True (or BASS_TRACE=1) crashes.  Registers the real
    ctypes NTFF hook when the axon .so is present, else a None-returning stub
    so tracing degrades gracefully.
    """
    import os
    import sys
    import types

    if "antenv.axon_hooks" in sys.modules:
        return
    try:
        import antenv
        import antenv.axon_hooks  # noqa: F401
        return  # real module exists
    except ImportError:
        pass
    mod = types.ModuleType("antenv.axon_hooks")
    state: dict = {"h": None}
    mod.set_axon_ntff_profile_hook = lambda h: state.__setitem__("h", h)
    mod.get_axon_ntff_profile_hook = lambda: state.get("h")
    sys.modules["antenv.axon_hooks"] = mod
    antenv.axon_hooks = mod
    so_path = "/opt/axon/libaxon_pjrt.so"
    if os.path.exists(so_path):
        try:
            from trn_agent_boot.trn_boot import _ntff_profile_via_ctypes

            state["h"] = _ntff_profile_via_ctypes(so_path)
        except Exception:
            pass


def _build_kernel_body(ctx: ExitStack, tc: tile.TileContext, t, use_biases):
    nc = tc.nc

    ones2 = t["ones2"].ap()
    zeros128 = t["zeros128"].ap()
    xT = t["xT"].ap().rearrange("(do p) s -> p do s", p=P)
    wqT = t["wqT"].ap().rearrange("(do p) c -> p do c", p=P)
    wkvT = t["wkvT"].ap().rearrange("(do p) c -> p do c", p=P)
    woT = t["woT"].ap().rearrange("(g p) e -> p g e", p=P)
    outT = t["outT"].ap().rearrange("(eo p) s -> p eo s", p=P)

    # ---- resident SBUF tensors (one pool, distinct tags = distinct slots) -
    const = ctx.enter_context(tc.tile_pool(name="const", bufs=1))

    def single(shape, name, dtype=F32):
        return const.tile(shape, dtype, name=name, tag=name)

    wkv_sb = single([P, NDT, 2 * CSL], "wkv_sb", F32R)
    wq_sb = single([P, NDT, CSL], "wq_sb", F32R)
    wo_sb = single([P, NG, D], "wo_sb", F32R)
    x_sb = single([P, NDT, S], "x_sb", F32R)
    qf_sb = single([P, NG, S], "qf_sb", F32R)
    if use_biases:
        bk_rep = single([P, CSL], "bk_rep")
        bv_rep = single([P, CSL], "bv_rep")
        bq_sb = single([P, NG], "bq_sb")
        bo_sb = single([P, NET], "bo_sb")

    # DMA issue order = compute-need order: K|V weights, first x chunk,
    # Q weights, the rest of x, then out weights / constants.
    nc.sync.dma_start(out=wkv_sb, in_=wkvT)
    nc.sync.dma_start(out=x_sb[:, :, 0:SC], in_=xT[:, :, 0:SC])
    nc.sync.dma_start(out=wq_sb, in_=wqT)
    for sc in range(1, NSC):
        nc.sync.dma_start(
            out=x_sb[:, :, sc * SC:(sc + 1) * SC],
            in_=xT[:, :, sc * SC:(sc + 1) * SC],
        )
    nc.sync.dma_start(out=wo_sb, in_=woT)
    if use_biases:
        nc.gpsimd.dma_start(out=bk_rep, in_=t["bk1"].ap().partition_broadcast(P))
        nc.gpsimd.dma_start(out=bv_rep, in_=t["bv1"].ap().partition_broadcast(P))
        nc.sync.dma_start(out=bq_sb, in_=t["bq2"].ap())
        nc.sync.dma_start(out=bo_sb, in_=t["bo8"].ap())

    # ---- phase A: K|V projection + feature map + KV/ksum accumulation,
    #      with Q projection interleaved per s-chunk -----------------------
    ps_kv = ctx.enter_context(tc.tile_pool(name="ps_kv", bufs=1, space="PSUM"))
    kv_ps = [ps_kv.tile([P, CSL + 2], F32, name=f"kv_ps{g}", tag=f"kv{g}")
             for g in range(NG)]

    with tc.tile_pool(name="ps_a", bufs=2, space="PSUM") as ps_a, \
         tc.tile_pool(name="ps_q", bufs=2, space="PSUM") as ps_q, \
         tc.tile_pool(name="sb_a", bufs=3) as sb_a:
        for sc in range(NSC):
            for sti in range(STC):
                st = sc * STC + sti
                ssl = slice(st * P, (st + 1) * P)
                # combined K|V projection: [s, 0:256]=K, [s, 256:512]=V
                kvp = ps_a.tile([P, 2 * CSL], F32, tag="kvp")
                for dt in range(NDT):
                    nc.tensor.matmul(
                        kvp, x_sb[:, dt, ssl], wkv_sb[:, dt, :],
                        start=(dt == 0), stop=(dt == NDT - 1))
                # kf = relu(K + bk) + exp(min(K + bk, 0))
                kf = sb_a.tile([P, CSL], F32R, tag="kf")
                m_k = sb_a.tile([P, CSL], F32, tag="m_k")
                if use_biases:
                    nc.vector.tensor_tensor(kf, kvp[:, 0:CSL], bk_rep, ALU.add)
                    nc.vector.tensor_scalar(m_k, kf, 0.0, None, op0=ALU.min)
                    nc.scalar.activation(m_k, m_k, AF.Exp)
                    nc.scalar.activation(kf, kf, AF.Relu)
                else:
                    nc.vector.tensor_scalar(
                        m_k, kvp[:, 0:CSL], 0.0, None, op0=ALU.min)
                    nc.scalar.activation(m_k, m_k, AF.Exp)
                    nc.scalar.activation(kf, kvp[:, 0:CSL], AF.Relu)
                nc.vector.tensor_tensor(kf, kf, m_k, ALU.add)

                # V (+bias) augmented with a ones column for ksum
                vaug = sb_a.tile([P, CSL + 2], F32R, tag="vaug")
                nc.sync.dma_start(out=vaug[:, CSL:CSL + 2], in_=ones2)
                if use_biases:
                    nc.vector.tensor_tensor(
                        vaug[:, 0:CSL], kvp[:, CSL:2 * CSL], bv_rep, ALU.add)
                else:
                    nc.scalar.copy(out=vaug[:, 0:CSL], in_=kvp[:, CSL:2 * CSL])

                # KV/ksum accumulation: kv_ps[g] += kf_g^T @ [V|1]
                for g in range(NG):
                    nc.tensor.matmul(
                        kv_ps[g], kf[:, g * P:(g + 1) * P], vaug,
                        start=(st == 0), stop=(st == NST - 1))

            # Q projection for this chunk (x already in SBUF; fills PE
            # while DMA streams the next chunk)
            csl = slice(sc * SC, (sc + 1) * SC)
            for g in range(NG):
                q_ps = ps_q.tile([P, SC], F32, tag="q_ps")
                for dt in range(NDT):
                    nc.tensor.matmul(
                        q_ps, wq_sb[:, dt, g * P:(g + 1) * P],
                        x_sb[:, dt, csl],
                        start=(dt == 0), stop=(dt == NDT - 1))
                m_q = sb_a.tile([P, SC], F32, tag="m_q")
                if use_biases:
                    nc.vector.tensor_scalar(
                        m_q, q_ps, bq_sb[:, g:g + 1], 0.0,
                        op0=ALU.add, op1=ALU.min)
                    nc.scalar.activation(m_q, m_q, AF.Exp)
                    nc.scalar.activation(
                        qf_sb[:, g, csl], q_ps, AF.Relu, bias=bq_sb[:, g:g + 1])
                else:
                    nc.vector.tensor_scalar(m_q, q_ps, 0.0, None, op0=ALU.min)
                    nc.scalar.activation(m_q, m_q, AF.Exp)
                    nc.scalar.activation(qf_sb[:, g, csl], q_ps, AF.Relu)
                nc.vector.tensor_tensor(
                    qf_sb[:, g, csl], qf_sb[:, g, csl], m_q, ALU.add)

    # ---- masked KV / ksum tiles ------------------------------------------
    # bkv[g][c',c] = KV[c',c] for head(c')==head(c) else 0 (block diagonal)
    # bden[g][c',c] = ksum[c'] for head(c')==head(c) else 0
    bkv = []
    bden = []
    for g in range(NG):
        bk_t = single([P, P], f"bkv{g}", F32R)
        bd_t = single([P, P], f"bden{g}", F32R)
        nc.sync.dma_start(out=bk_t, in_=zeros128)
        nc.sync.dma_start(out=bd_t, in_=zeros128)
        for hb in range(2):
            hsl = slice(hb * 64, (hb + 1) * 64)
            csl = slice(g * P + hb * 64, g * P + (hb + 1) * 64)
            nc.vector.tensor_copy(out=bk_t[hsl, hsl], in_=kv_ps[g][hsl, csl])
            nc.vector.tensor_copy(
                out=bd_t[hsl, hsl],
                in_=kv_ps[g][hsl, CSL:CSL + 1].to_broadcast((64, 64)))
        bkv.append(bk_t)
        bden.append(bd_t)

    # ---- attention + output projection per s-chunk -----------------------
    with tc.tile_pool(name="ps_b", bufs=2, space="PSUM") as ps_b, \
         tc.tile_pool(name="sb_b", bufs=2) as sb_b, \
         tc.tile_pool(name="sb_o", bufs=3) as sb_o:
        for sc in range(NSC):
            csl = slice(sc * SC, (sc + 1) * SC)
            att = []
            for g in range(NG):
                # den_rep[c,s] = sum_{c'} bden[c',c] qf[c',s] (head-replicated)
                d_ps = ps_b.tile([P, SC], F32, tag="d_ps")
                nc.tensor.matmul(d_ps, bden[g], qf_sb[:, g, csl],
                                 start=True, stop=True)
                rden = sb_b.tile([P, SC], F32, tag="rden")
                nc.vector.tensor_scalar(rden, d_ps, EPS, None, op0=ALU.max)
                nc.vector.reciprocal(rden, rden)
                qs = sb_b.tile([P, SC], F32R, tag="qs")
                nc.vector.tensor_mul(qs, qf_sb[:, g, csl], rden)
                # attT[c,s] = sum_{c'} bkv[c',c] qs[c',s]
                a_ps = ps_b.tile([P, SC], F32, tag="a_ps")
                nc.tensor.matmul(a_ps, bkv[g], qs, start=True, stop=True)
                a_sb = sb_b.tile([P, SC], F32R, tag=f"att{g}")
                nc.scalar.copy(out=a_sb, in_=a_ps)
                att.append(a_sb)

            for et in range(NET):
                esl = slice(et * P, (et + 1) * P)
                o_ps = ps_b.tile([P, SC], F32, tag="o_ps")
                for g in range(NG):
                    nc.tensor.matmul(o_ps, wo_sb[:, g, esl], att[g],
                                     start=(g == 0), stop=(g == NG - 1))
                o_sb = sb_o.tile([P, SC], F32, tag="o_sb")
                if use_biases:
                    if et % 2 == 0:
                        nc.vector.tensor_scalar(
                            o_sb, o_ps, bo_sb[:, et:et + 1], None, op0=ALU.add)
                    else:
                        nc.scalar.activation(
                            o_sb, o_ps, AF.Identity, bias=bo_sb[:, et:et + 1])
                else:
                    if et % 2 == 0:
                        nc.vector.tensor_copy(out=o_sb, in_=o_ps)
                    else:
                        nc.scalar.copy(out=o_sb, in_=o_ps)
                nc.sync.dma_start(out=outT[:, et, csl], in_=o_sb)


def build_nc(use_biases):
    nc = bacc.Bacc("TRN2", target_bir_lowering=False, debug=False)
    t = {}
    t["xT"] = nc.dram_tensor("xT", [D, S], F32R, kind="ExternalInput")
    t["wqT"] = nc.dram_tensor("wqT", [D, CSL], F32R, kind="ExternalInput")
    t["wkvT"] = nc.dram_tensor("wkvT", [D, 2 * CSL], F32R, kind="ExternalInput")
    t["woT"] = nc.dram_tensor("woT", [CSL, D], F32R, kind="ExternalInput")
    if use_biases:
        t["bq2"] = nc.dram_tensor("bq2", [P, NG], F32, kind="ExternalInput")
        t["bk1"] = nc.dram_tensor("bk1", [CSL], F32, kind="ExternalInput")
        t["bv1"] = nc.dram_tensor("bv1", [CSL], F32, kind="ExternalInput")
        t["bo8"] = nc.dram_tensor("bo8", [P, NET], F32, kind="ExternalInput")
    t["ones2"] = nc.dram_tensor("ones2", [P, 2], F32R, kind="ExternalInput")
    t["zeros128"] = nc.dram_tensor("zeros128", [P, P], F32R, kind="ExternalInput")
    t["outT"] = nc.dram_tensor("outT", [D, S], F32, kind="ExternalOutput")

    with tile.TileContext(nc) as tc:
        with ExitStack() as ctx:
            _build_kernel_body(ctx, tc, t, use_biases)
    nc.compile()
    return nc


def _get_nc(use_biases):
    key = ("nc", use_biases)
    if key not in _CACHE:
        _CACHE[key] = build_nc(use_biases)
    return _CACHE[key]


def make_in_maps(x, wq, bq, wk, bk, wv, bv, wo, bo, use_biases=None):
    """Shard the full inputs into the 8 per-core input maps."""
    f = lambda a: np.ascontiguousarray(np.asarray(a), dtype=np.float32)
    x, wq, bq, wk, bk = f(x), f(wq), f(bq), f(wk), f(bk)
    wv, bv, wo, bo = f(wv), f(bv), f(wo), f(bo)
    if use_biases is None:
        use_biases = any(np.any(b) for b in (bq, bk, bv, bo))
    in_maps = []
    for cid in range(8):
        b, hg = divmod(cid, 4)
        hs = slice(hg * CSL, (hg + 1) * CSL)
        m = {
            "xT": np.ascontiguousarray(x[b].T),
            "wqT": np.ascontiguousarray(wq[hs, :].T),
            "wkvT": np.ascontiguousarray(
                np.concatenate([wk[hs, :].T, wv[hs, :].T], axis=1)),
            "woT": np.ascontiguousarray(wo[:, hs].T),
            "ones2": np.ascontiguousarray(
                np.tile(np.array([1.0, 0.0], np.float32), (P, 1))),
            "zeros128": np.zeros((P, P), np.float32),
        }
        if use_biases:
            bo_in = bo if hg == 0 else np.zeros_like(bo)
            m["bq2"] = np.ascontiguousarray(bq[hs].reshape(NG, P).T)
            m["bk1"] = bk[hs].copy()
            m["bv1"] = bv[hs].copy()
            m["bo8"] = np.ascontiguousarray(bo_in.reshape(NET, P).T)
        in_maps.append(m)
    return in_maps, use_biases


def unshard(results):
    """Sum head-group partials per batch and undo the output transpose."""
    out = np.zeros((BATCH, S, D), np.float32)
    for cid in range(8):
        b = cid // 4
        out[b] += results[cid]["outT"].T
    return out


def kernel(x, wq, bq, wk, bk, wv, bv, wo, bo):
    in_maps, use_biases = make_in_maps(x, wq, bq, wk, bk, wv, bv, wo, bo)
    nc = _get_nc(use_biases)
    res = bass_utils.run_bass_kernel_spmd(nc, in_maps, core_ids=list(range(8)))
    return unshard(res.results)



# revision 4
# speedup vs baseline: 1.2332x; 1.2332x over previous
"""Trainium2 Bass kernel for ConvexLinearAttention (elu(x)+1 linear attention).

Full-input contract: kernel(**inputs) takes the unsharded tensors
(x [2,2048,1024], wq/wk/wv/wo [1024,1024], bq/bk/bv/bo [1024]) and returns the
full output [2,2048,1024].

Sharding (8 cores): data-parallel over batch (2) x head-group-parallel (4 groups
of 4 heads).  Each core projects only its 256-wide head slice, runs the
linearized attention (attended = qf @ (kf^T V) / (qf @ sum(kf)) -- an exact
refactoring of the dense normalized scores), and emits a partial output
projection.  The host sums the 4 head-group partials per batch (the
tensor-parallel unshard).

All matmul operands are bf16 (fp32 PSUM accumulate): bf16 stationary tiles get
fast-weight-load + background-buffer overlap so LDWEIGHTS hides under the
previous matmul (fp32r stationary serializes them), and bf16 halves every DMA
transfer.  The elu(x)+1 feature map is relu(x) + exp(min(x,0)) in three ops
(vector min, scalar exp, fused vector relu+add).  Attention is computed
unnormalized (a = bkv @ qf, d = bden @ qf back-to-back on the PE) and
normalized afterwards on the vector engine (att = a * 1/d); the max(d, EPS)
clamp is dropped because qf,kf > 0 makes d >= O(1e4) mathematically.
"""

from contextlib import ExitStack

import ml_dtypes
import numpy as np

import concourse.bass as bass
import concourse.mybir as mybir
import concourse.tile as tile
from concourse import bacc, bass_utils

F32 = mybir.dt.float32
BF16 = mybir.dt.bfloat16
AF = mybir.ActivationFunctionType
ALU = mybir.AluOpType

D = 1024          # model dim
S = 2048          # sequence length
BATCH = 2
CSL = 256         # head-slice width per core (4 heads x 64)
NG = 2            # 128-wide c-groups per core
P = 128
NDT = D // P      # 8 d-tiles
NST = S // P      # 16 s-tiles
SC = 512          # s-chunk (matmul moving dim)
NSC = S // SC     # 4 s-chunks
STC = SC // P     # 4 s-tiles per chunk
NET = D // P      # 8 e-tiles

_CACHE: dict = {}


def install_ntff_hook_shim():
    """Provide ``antenv.axon_hooks`` when the image ships only the antenv stub.

    concourse.bass_utils imports it unconditionally on the axon trace path;
    without this shim trace=True (or BASS_TRACE=1) crashes.  Registers the real
    ctypes NTFF hook when the axon .so is present, else a None-returning stub
    so tracing degrades gracefully.
    """
    import os
    import sys
    import types

    if "antenv.axon_hooks" in sys.modules:
        return
    try:
        import antenv
        import antenv.axon_hooks  # noqa: F401
        return  # real module exists
    except ImportError:
        pass
    mod = types.ModuleType("antenv.axon_hooks")
    state: dict = {"h": None}
    mod.set_axon_ntff_profile_hook = lambda h: state.__setitem__("h", h)
    mod.get_axon_ntff_profile_hook = lambda: state.get("h")
    sys.modules["antenv.axon_hooks"] = mod
    antenv.axon_hooks = mod
    so_path = "/opt/axon/libaxon_pjrt.so"
    if os.path.exists(so_path):
        try:
            from trn_agent_boot.trn_boot import _ntff_profile_via_ctypes

            state["h"] = _ntff_profile_via_ctypes(so_path)
        except Exception:
            pass


def _build_kernel_body(ctx: ExitStack, tc: tile.TileContext, t, use_biases):
    nc = tc.nc

    xT = t["xT"].ap().rearrange("(do p) s -> p do s", p=P)
    wqT = t["wqT"].ap().rearrange("(do p) c -> p do c", p=P)
    wkvT = t["wkvT"].ap().rearrange("(do p) c -> p do c", p=P)
    woT = t["woT"].ap().rearrange("(g p) e -> p g e", p=P)
    outT = t["outT"].ap().rearrange("(eo p) s -> p eo s", p=P)

    # ---- resident SBUF tensors (one pool, distinct tags = distinct slots) -
    const = ctx.enter_context(tc.tile_pool(name="const", bufs=1))

    def single(shape, name, dtype=F32):
        return const.tile(shape, dtype, name=name, tag=name)

    wkv_sb = single([P, NDT, 2 * CSL], "wkv_sb", BF16)
    wq_sb = single([P, NDT, CSL], "wq_sb", BF16)
    wo_sb = single([P, NG, D], "wo_sb", BF16)
    x_sb = single([P, NDT, S], "x_sb", BF16)
    qf_sb = single([P, NG, S], "qf_sb", BF16)
    if use_biases:
        bk_rep = single([P, CSL], "bk_rep")
        bv_rep = single([P, CSL], "bv_rep")
        bq_sb = single([P, NG], "bq_sb")
        bo_sb = single([P, NET], "bo_sb")

    # DMA issue order = compute-need order: K|V weights, first x chunk (in two
    # halves so the first matmuls can start sooner), Q weights, the rest of x,
    # then out weights / biases.
    nc.sync.dma_start(out=wkv_sb, in_=wkvT)
    nc.sync.dma_start(out=x_sb[:, :, 0:SC // 2], in_=xT[:, :, 0:SC // 2])
    nc.sync.dma_start(out=x_sb[:, :, SC // 2:SC], in_=xT[:, :, SC // 2:SC])
    nc.sync.dma_start(out=wq_sb, in_=wqT)
    for sc in range(1, NSC):
        nc.sync.dma_start(
            out=x_sb[:, :, sc * SC:(sc + 1) * SC],
            in_=xT[:, :, sc * SC:(sc + 1) * SC],
        )
    nc.sync.dma_start(out=wo_sb, in_=woT)
    if use_biases:
        nc.gpsimd.dma_start(out=bk_rep, in_=t["bk1"].ap().partition_broadcast(P))
        nc.gpsimd.dma_start(out=bv_rep, in_=t["bv1"].ap().partition_broadcast(P))
        nc.sync.dma_start(out=bq_sb, in_=t["bq2"].ap())
        nc.sync.dma_start(out=bo_sb, in_=t["bo8"].ap())

    # bkv[g][c',c] = KV[c',c] for head(c')==head(c) else 0 (block diagonal)
    # bden[g][c',c] = ksum[c'] for head(c')==head(c) else 0
    bkv = [single([P, P], f"bkv{g}", BF16) for g in range(NG)]
    bden = [single([P, P], f"bden{g}", BF16) for g in range(NG)]
    for g in range(NG):
        nc.gpsimd.memset(bkv[g], 0.0)
        nc.gpsimd.memset(bden[g], 0.0)

    # ---- phase A: K|V projection + feature map + KV/ksum accumulation,
    #      with Q projection interleaved per s-chunk -----------------------
    with tc.tile_pool(name="ps_kv", bufs=1, space="PSUM") as ps_kv, \
         tc.tile_pool(name="ps_a", bufs=2, space="PSUM") as ps_a, \
         tc.tile_pool(name="ps_q", bufs=2, space="PSUM") as ps_q, \
         tc.tile_pool(name="sb_a", bufs=3) as sb_a:
        kv_ps = [ps_kv.tile([P, CSL + 2], F32, name=f"kv_ps{g}", tag=f"kv{g}")
                 for g in range(NG)]
        for sc in range(NSC):
            for sti in range(STC):
                st = sc * STC + sti
                ssl = slice(st * P, (st + 1) * P)
                # combined K|V projection: [s, 0:256]=K, [s, 256:512]=V
                kvp = ps_a.tile([P, 2 * CSL], F32, tag="kvp")
                for dt in range(NDT):
                    nc.tensor.matmul(
                        kvp, x_sb[:, dt, ssl], wkv_sb[:, dt, :],
                        start=(dt == 0), stop=(dt == NDT - 1))
                # kf = relu(K + bk) + exp(min(K + bk, 0))
                kf = sb_a.tile([P, CSL], BF16, tag="kf")
                m_k = sb_a.tile([P, CSL], BF16, tag="m_k")
                # V (+bias) augmented with a ones column for ksum
                vaug = sb_a.tile([P, CSL + 2], BF16, tag="vaug")
                nc.gpsimd.memset(vaug[:, CSL:CSL + 2], 1.0)
                if use_biases:
                    kb = sb_a.tile([P, CSL], BF16, tag="kb")
                    nc.vector.tensor_tensor(kb, kvp[:, 0:CSL], bk_rep, ALU.add)
                    nc.gpsimd.tensor_scalar_min(m_k, kb, 0.0)
                    nc.scalar.activation(m_k, m_k, AF.Exp)
                    nc.gpsimd.scalar_tensor_tensor(
                        kf, kb, 0.0, m_k, op0=ALU.max, op1=ALU.add)
                    nc.vector.tensor_tensor(
                        vaug[:, 0:CSL], kvp[:, CSL:2 * CSL], bv_rep, ALU.add)
                else:
                    nc.vector.tensor_scalar_min(m_k, kvp[:, 0:CSL], 0.0)
                    nc.scalar.activation(m_k, m_k, AF.Exp)
                    nc.vector.scalar_tensor_tensor(
                        kf, kvp[:, 0:CSL], 0.0, m_k, op0=ALU.max, op1=ALU.add)
                    nc.scalar.copy(out=vaug[:, 0:CSL], in_=kvp[:, CSL:2 * CSL])

                # KV/ksum accumulation: kv_ps[g] += kf_g^T @ [V|1]
                for g in range(NG):
                    nc.tensor.matmul(
                        kv_ps[g], kf[:, g * P:(g + 1) * P], vaug,
                        start=(st == 0), stop=(st == NST - 1))

            # After the last KV accumulation, build the masked KV/ksum
            # stationary tiles on the vector engine while the PE runs the
            # last chunk's Q projection.
            if sc == NSC - 1:
                for g in range(NG):
                    for hb in range(2):
                        hsl = slice(hb * 64, (hb + 1) * 64)
                        csl2 = slice(g * P + hb * 64, g * P + (hb + 1) * 64)
                        nc.vector.tensor_copy(
                            out=bkv[g][hsl, hsl], in_=kv_ps[g][hsl, csl2])
                        nc.vector.tensor_copy(
                            out=bden[g][hsl, hsl],
                            in_=kv_ps[g][hsl, CSL:CSL + 1].to_broadcast((64, 64)))

            # Q projection for this chunk (x already in SBUF; fills PE
            # while DMA streams the next chunk)
            csl = slice(sc * SC, (sc + 1) * SC)
            for g in range(NG):
                q_ps = ps_q.tile([P, SC], F32, tag="q_ps")
                for dt in range(NDT):
                    nc.tensor.matmul(
                        q_ps, wq_sb[:, dt, g * P:(g + 1) * P],
                        x_sb[:, dt, csl],
                        start=(dt == 0), stop=(dt == NDT - 1))
                m_q = sb_a.tile([P, SC], BF16, tag="m_q")
                if use_biases:
                    nc.vector.tensor_scalar(
                        m_q, q_ps, bq_sb[:, g:g + 1], 0.0,
                        op0=ALU.add, op1=ALU.min)
                    nc.scalar.activation(m_q, m_q, AF.Exp)
                    rq = sb_a.tile([P, SC], BF16, tag="rq")
                    nc.scalar.activation(
                        rq, q_ps, AF.Relu, bias=bq_sb[:, g:g + 1])
                    nc.gpsimd.tensor_tensor(
                        qf_sb[:, g, csl], rq, m_q, ALU.add)
                else:
                    nc.vector.tensor_scalar_min(m_q, q_ps, 0.0)
                    nc.scalar.activation(m_q, m_q, AF.Exp)
                    nc.vector.scalar_tensor_tensor(
                        qf_sb[:, g, csl], q_ps, 0.0, m_q,
                        op0=ALU.max, op1=ALU.add)

    # ---- phase B: attention + output projection per s-chunk --------------
    # a = bkv @ qf and d = bden @ qf issue back-to-back on the PE; the
    # normalization att = a * (1/d) runs on vector afterwards, so the PE
    # never waits on the vector chain.
    with tc.tile_pool(name="ps_b", bufs=1, space="PSUM") as ps_b, \
         tc.tile_pool(name="ps_o", bufs=2, space="PSUM") as ps_o, \
         tc.tile_pool(name="sb_b", bufs=2) as sb_b, \
         tc.tile_pool(name="sb_o", bufs=3) as sb_o:
        for sc in range(NSC):
            csl = slice(sc * SC, (sc + 1) * SC)
            att = []
            for g in range(NG):
                d_ps = ps_b.tile([P, SC], F32, tag=f"d_ps{g}")
                nc.tensor.matmul(d_ps, bden[g], qf_sb[:, g, csl],
                                 start=True, stop=True)
                a_ps = ps_b.tile([P, SC], F32, tag=f"a_ps{g}")
                nc.tensor.matmul(a_ps, bkv[g], qf_sb[:, g, csl],
                                 start=True, stop=True)
                rden = sb_b.tile([P, SC], F32, tag=f"rden{g}")
                nc.vector.reciprocal(rden, d_ps)
                a_sb = sb_b.tile([P, SC], BF16, tag=f"att{g}")
                nc.vector.tensor_tensor(a_sb, a_ps, rden, ALU.mult)
                att.append(a_sb)

            for et in range(NET):
                esl = slice(et * P, (et + 1) * P)
                o_ps = ps_o.tile([P, SC], F32, tag="o_ps")
                for g in range(NG):
                    nc.tensor.matmul(o_ps, wo_sb[:, g, esl], att[g],
                                     start=(g == 0), stop=(g == NG - 1))
                o_sb = sb_o.tile([P, SC], BF16, tag="o_sb")
                if use_biases:
                    if et % 2 == 0:
                        nc.vector.tensor_scalar(
                            o_sb, o_ps, bo_sb[:, et:et + 1], None, op0=ALU.add)
                    else:
                        nc.scalar.activation(
                            o_sb, o_ps, AF.Identity, bias=bo_sb[:, et:et + 1])
                else:
                    if et % 2 == 0:
                        nc.vector.tensor_copy(out=o_sb, in_=o_ps)
                    else:
                        nc.scalar.copy(out=o_sb, in_=o_ps)
                nc.sync.dma_start(out=outT[:, et, csl], in_=o_sb)


def build_nc(use_biases):
    nc = bacc.Bacc("TRN2", target_bir_lowering=False, debug=False)
    t = {}
    t["xT"] = nc.dram_tensor("xT", [D, S], BF16, kind="ExternalInput")
    t["wqT"] = nc.dram_tensor("wqT", [D, CSL], BF16, kind="ExternalInput")
    t["wkvT"] = nc.dram_tensor("wkvT", [D, 2 * CSL], BF16, kind="ExternalInput")
    t["woT"] = nc.dram_tensor("woT", [CSL, D], BF16, kind="ExternalInput")
    if use_biases:
        t["bq2"] = nc.dram_tensor("bq2", [P, NG], F32, kind="ExternalInput")
        t["bk1"] = nc.dram_tensor("bk1", [CSL], F32, kind="ExternalInput")
        t["bv1"] = nc.dram_tensor("bv1", [CSL], F32, kind="ExternalInput")
        t["bo8"] = nc.dram_tensor("bo8", [P, NET], F32, kind="ExternalInput")
    t["outT"] = nc.dram_tensor("outT", [D, S], BF16, kind="ExternalOutput")

    with tile.TileContext(nc) as tc:
        with ExitStack() as ctx:
            _build_kernel_body(ctx, tc, t, use_biases)
    nc.compile()
    return nc


def _get_nc(use_biases):
    key = ("nc", use_biases)
    if key not in _CACHE:
        _CACHE[key] = build_nc(use_biases)
    return _CACHE[key]


def make_in_maps(x, wq, bq, wk, bk, wv, bv, wo, bo, use_biases=None):
    """Shard the full inputs into the 8 per-core input maps."""
    f = lambda a: np.asarray(a, dtype=np.float32)
    x, wq, bq, wk, bk = f(x), f(wq), f(bq), f(wk), f(bk)
    wv, bv, wo, bo = f(wv), f(bv), f(wo), f(bo)
    bf = lambda a: np.ascontiguousarray(a).astype(ml_dtypes.bfloat16)
    if use_biases is None:
        use_biases = any(np.any(b) for b in (bq, bk, bv, bo))
    in_maps = []
    for cid in range(8):
        b, hg = divmod(cid, 4)
        hs = slice(hg * CSL, (hg + 1) * CSL)
        m = {
            "xT": bf(x[b].T),
            "wqT": bf(wq[hs, :].T),
            "wkvT": bf(np.concatenate([wk[hs, :].T, wv[hs, :].T], axis=1)),
            "woT": bf(wo[:, hs].T),
        }
        if use_biases:
            bo_in = bo if hg == 0 else np.zeros_like(bo)
            m["bq2"] = np.ascontiguousarray(bq[hs].reshape(NG, P).T)
            m["bk1"] = bk[hs].copy()
            m["bv1"] = bv[hs].copy()
            m["bo8"] = np.ascontiguousarray(bo_in.reshape(NET, P).T)
        in_maps.append(m)
    return in_maps, use_biases


def unshard(results):
    """Sum head-group partials per batch and undo the output transpose."""
    out = np.zeros((BATCH, S, D), np.float32)
    for cid in range(8):
        b = cid // 4
        out[b] += results[cid]["outT"].T.astype(np.float32)
    return out


def kernel(x, wq, bq, wk, bk, wv, bv, wo, bo):
    in_maps, use_biases = make_in_maps(x, wq, bq, wk, bk, wv, bv, wo, bo)
    nc = _get_nc(use_biases)
    res = bass_utils.run_bass_kernel_spmd(nc, in_maps, core_ids=list(range(8)))
    return unshard(res.results)


# revision 11
# speedup vs baseline: 1.5490x; 1.2561x over previous
"""Trainium2 Bass kernel for ConvexLinearAttention (elu(x)+1 linear attention).

Full-input contract: kernel(**inputs) takes the unsharded tensors
(x [2,2048,1024], wq/wk/wv/wo [1024,1024], bq/bk/bv/bo [1024]) and returns the
full output [2,2048,1024].

Sharding (8 cores): data-parallel over batch (2) x head-group-parallel (4 groups
of 4 heads).  Each core projects only its 256-wide head slice, runs the
linearized attention (attended = qf @ (kf^T V) / (qf @ sum(kf)) -- an exact
refactoring of the dense normalized scores), and emits a partial output
projection.  The host sums the 4 head-group partials per batch (the
tensor-parallel unshard).

All matmul operands are bf16 (fp32 PSUM accumulate): bf16 stationary tiles get
fast-weight-load + background-buffer overlap so LDWEIGHTS hides under the
previous matmul (fp32r stationary serializes them), and bf16 halves every DMA
transfer.  The elu(x)+1 feature map is relu(x) + exp(min(x,0)) in three ops
(vector min, scalar exp, fused vector relu+add).  Attention is computed
unnormalized (a = bkv @ qf, d = bden @ qf back-to-back on the PE) and
normalized afterwards on the vector engine (att = a * 1/d); the max(d, EPS)
clamp is dropped because qf,kf > 0 makes d >= O(1e4) mathematically.
"""

from contextlib import ExitStack

import ml_dtypes
import numpy as np

import concourse.bass as bass
import concourse.mybir as mybir
import concourse.tile as tile
from concourse import bacc, bass_utils

F32 = mybir.dt.float32
BF16 = mybir.dt.bfloat16
AF = mybir.ActivationFunctionType
ALU = mybir.AluOpType

D = 1024          # model dim
S = 2048          # sequence length
BATCH = 2
CSL = 256         # head-slice width per core (4 heads x 64)
NG = 2            # 128-wide c-groups per core
P = 128
NDT = D // P      # 8 d-tiles
NST = S // P      # 16 s-tiles
SC = 512          # s-chunk (matmul moving dim)
NSC = S // SC     # 4 s-chunks
STC = SC // P     # 4 s-tiles per chunk
NET = D // P      # 8 e-tiles

_CACHE: dict = {}


def install_ntff_hook_shim():
    """Provide ``antenv.axon_hooks`` when the image ships only the antenv stub.

    concourse.bass_utils imports it unconditionally on the axon trace path;
    without this shim trace=True (or BASS_TRACE=1) crashes.  Registers the real
    ctypes NTFF hook when the axon .so is present, else a None-returning stub
    so tracing degrades gracefully.
    """
    import os
    import sys
    import types

    if "antenv.axon_hooks" in sys.modules:
        return
    try:
        import antenv
        import antenv.axon_hooks  # noqa: F401
        return  # real module exists
    except ImportError:
        pass
    mod = types.ModuleType("antenv.axon_hooks")
    state: dict = {"h": None}
    mod.set_axon_ntff_profile_hook = lambda h: state.__setitem__("h", h)
    mod.get_axon_ntff_profile_hook = lambda: state.get("h")
    sys.modules["antenv.axon_hooks"] = mod
    antenv.axon_hooks = mod
    so_path = "/opt/axon/libaxon_pjrt.so"
    if os.path.exists(so_path):
        try:
            from trn_agent_boot.trn_boot import _ntff_profile_via_ctypes

            state["h"] = _ntff_profile_via_ctypes(so_path)
        except Exception:
            pass


def _build_kernel_body(ctx: ExitStack, tc: tile.TileContext, t, use_biases):
    nc = tc.nc

    # Host-side pre-swizzled layouts: every dram tensor is per-partition
    # contiguous so each DMA is 128 large descriptors instead of ~1k small
    # ones (descriptor processing dominates the startup otherwise).
    xT = t["xT"].ap().rearrange("p (h do s) -> p h do s", do=NDT, s=SC // 2)
    wqT = t["wqT"].ap().rearrange("p (do c) -> p do c", do=NDT)
    wkvT = t["wkvT"].ap().rearrange("p (do c) -> p do c", do=NDT)
    woT = t["woT"].ap().rearrange("p (g e) -> p g e", g=NG)
    outT = t["outT"].ap().rearrange("(eo p) s -> p eo s", p=P)

    # ---- resident SBUF tensors (one pool, distinct tags = distinct slots) -
    const = ctx.enter_context(tc.tile_pool(name="const", bufs=1))

    def single(shape, name, dtype=F32):
        return const.tile(shape, dtype, name=name, tag=name)

    wkv_sb = single([P, NDT, 2 * CSL], "wkv_sb", BF16)
    wq_sb = single([P, NDT, CSL], "wq_sb", BF16)
    wo_sb = single([P, NG, D], "wo_sb", BF16)
    x_sb = single([P, NDT, S], "x_sb", BF16)
    qf_sb = single([P, NG, S], "qf_sb", BF16)
    if use_biases:
        bk_rep = single([P, CSL], "bk_rep")
        bv_rep = single([P, CSL], "bv_rep")
        bq_sb = single([P, NG], "bq_sb")
        bo_sb = single([P, NET], "bo_sb")

    # DMA issue order = compute-need order: K|V weights, first x half-chunks
    # (so the first matmuls start sooner), Q weights, the rest of x, then out
    # weights / biases.  x arrives in 512 KB half-chunk transfers whose dram
    # and SBUF access patterns match element-for-element.
    HC = SC // 2
    nc.sync.dma_start(out=wkv_sb, in_=wkvT)
    nc.sync.dma_start(out=x_sb[:, :, 0:HC], in_=xT[:, 0])
    nc.sync.dma_start(out=x_sb[:, :, HC:2 * HC], in_=xT[:, 1])
    nc.sync.dma_start(out=wq_sb, in_=wqT)
    for h in range(2, 2 * NSC):
        nc.sync.dma_start(out=x_sb[:, :, h * HC:(h + 1) * HC], in_=xT[:, h])
    nc.sync.dma_start(out=wo_sb, in_=woT)
    if use_biases:
        nc.gpsimd.dma_start(out=bk_rep, in_=t["bk1"].ap().partition_broadcast(P))
        nc.gpsimd.dma_start(out=bv_rep, in_=t["bv1"].ap().partition_broadcast(P))
        nc.sync.dma_start(out=bq_sb, in_=t["bq2"].ap())
        nc.sync.dma_start(out=bo_sb, in_=t["bo8"].ap())

    # bkv[g][c',c] = KV[c',c] for head(c')==head(c) else 0 (block diagonal)
    # bden[g][c',c] = ksum[c'] for head(c')==head(c) else 0
    bkv = [single([P, P], f"bkv{g}", BF16) for g in range(NG)]
    bden = [single([P, P], f"bden{g}", BF16) for g in range(NG)]
    for g in range(NG):
        nc.gpsimd.memset(bkv[g], 0.0)
        nc.gpsimd.memset(bden[g], 0.0)

    # ---- phase A: K|V projection + feature map + KV/ksum accumulation,
    #      with Q projection interleaved per s-chunk -----------------------
    with tc.tile_pool(name="ps_kv", bufs=1, space="PSUM") as ps_kv, \
         tc.tile_pool(name="ps_a", bufs=3, space="PSUM") as ps_a, \
         tc.tile_pool(name="ps_q", bufs=2, space="PSUM") as ps_q, \
         tc.tile_pool(name="sb_a", bufs=4) as sb_a:
        kv_ps = [ps_kv.tile([P, CSL + 2], F32, name=f"kv_ps{g}", tag=f"kv{g}")
                 for g in range(NG)]
        for sc in range(NSC):
            for sti in range(STC):
                st = sc * STC + sti
                ssl = slice(st * P, (st + 1) * P)
                # combined K|V projection: [s, 0:256]=K, [s, 256:512]=V
                kvp = ps_a.tile([P, 2 * CSL], F32, tag="kvp")
                for dt in range(NDT):
                    nc.tensor.matmul(
                        kvp, x_sb[:, dt, ssl], wkv_sb[:, dt, :],
                        start=(dt == 0), stop=(dt == NDT - 1))
                # kf = relu(K + bk) + exp(min(K + bk, 0))
                kf = sb_a.tile([P, CSL], BF16, tag="kf")
                m_k = sb_a.tile([P, CSL], BF16, tag="m_k")
                # V (+bias) augmented with a ones column for ksum
                vaug = sb_a.tile([P, CSL + 2], BF16, tag="vaug")
                nc.gpsimd.memset(vaug[:, CSL:CSL + 2], 1.0)
                if use_biases:
                    kb = sb_a.tile([P, CSL], BF16, tag="kb")
                    nc.vector.tensor_tensor(kb, kvp[:, 0:CSL], bk_rep, ALU.add)
                    nc.gpsimd.tensor_scalar_min(m_k, kb, 0.0)
                    nc.scalar.activation(m_k, m_k, AF.Exp)
                    nc.gpsimd.scalar_tensor_tensor(
                        kf, kb, 0.0, m_k, op0=ALU.max, op1=ALU.add)
                    nc.vector.tensor_tensor(
                        vaug[:, 0:CSL], kvp[:, CSL:2 * CSL], bv_rep, ALU.add)
                else:
                    nc.vector.tensor_scalar_min(m_k, kvp[:, 0:CSL], 0.0)
                    nc.scalar.activation(m_k, m_k, AF.Exp)
                    nc.vector.scalar_tensor_tensor(
                        kf, kvp[:, 0:CSL], 0.0, m_k, op0=ALU.max, op1=ALU.add)
                    nc.scalar.copy(out=vaug[:, 0:CSL], in_=kvp[:, CSL:2 * CSL])

                # KV/ksum accumulation: kv_ps[g] += kf_g^T @ [V|1]
                for g in range(NG):
                    nc.tensor.matmul(
                        kv_ps[g], kf[:, g * P:(g + 1) * P], vaug,
                        start=(st == 0), stop=(st == NST - 1))

            # After the last KV accumulation, build the masked KV/ksum
            # stationary tiles on the vector engine while the PE runs the
            # last chunk's Q projection.
            if sc == NSC - 1:
                for g in range(NG):
                    for hb in range(2):
                        hsl = slice(hb * 64, (hb + 1) * 64)
                        csl2 = slice(g * P + hb * 64, g * P + (hb + 1) * 64)
                        nc.vector.tensor_copy(
                            out=bkv[g][hsl, hsl], in_=kv_ps[g][hsl, csl2])
                        nc.vector.tensor_copy(
                            out=bden[g][hsl, hsl],
                            in_=kv_ps[g][hsl, CSL:CSL + 1].to_broadcast((64, 64)))

            # Q projection for this chunk (x already in SBUF; fills PE
            # while DMA streams the next chunk)
            csl = slice(sc * SC, (sc + 1) * SC)
            for g in range(NG):
                q_ps = ps_q.tile([P, SC], F32, tag="q_ps")
                for dt in range(NDT):
                    nc.tensor.matmul(
                        q_ps, wq_sb[:, dt, g * P:(g + 1) * P],
                        x_sb[:, dt, csl],
                        start=(dt == 0), stop=(dt == NDT - 1))
                m_q = sb_a.tile([P, SC], BF16, tag="m_q")
                if use_biases:
                    nc.vector.tensor_scalar(
                        m_q, q_ps, bq_sb[:, g:g + 1], 0.0,
                        op0=ALU.add, op1=ALU.min)
                    nc.scalar.activation(m_q, m_q, AF.Exp)
                    rq = sb_a.tile([P, SC], BF16, tag="rq")
                    nc.scalar.activation(
                        rq, q_ps, AF.Relu, bias=bq_sb[:, g:g + 1])
                    nc.gpsimd.tensor_tensor(
                        qf_sb[:, g, csl], rq, m_q, ALU.add)
                else:
                    nc.vector.tensor_scalar_min(m_q, q_ps, 0.0)
                    nc.scalar.activation(m_q, m_q, AF.Exp)
                    nc.vector.scalar_tensor_tensor(
                        qf_sb[:, g, csl], q_ps, 0.0, m_q,
                        op0=ALU.max, op1=ALU.add)

    # ---- phase B: attention + output projection per s-chunk --------------
    # a = bkv @ qf and d = bden @ qf issue back-to-back on the PE; the
    # normalization att = a * (1/d) runs on vector afterwards, so the PE
    # never waits on the vector chain.
    with tc.tile_pool(name="ps_b", bufs=1, space="PSUM") as ps_b, \
         tc.tile_pool(name="ps_o", bufs=3, space="PSUM") as ps_o, \
         tc.tile_pool(name="sb_b", bufs=2) as sb_b, \
         tc.tile_pool(name="sb_o", bufs=4) as sb_o:
        for sc in range(NSC):
            csl = slice(sc * SC, (sc + 1) * SC)
            att = []
            for g in range(NG):
                d_ps = ps_b.tile([P, SC], F32, tag=f"d_ps{g}")
                nc.tensor.matmul(d_ps, bden[g], qf_sb[:, g, csl],
                                 start=True, stop=True)
                a_ps = ps_b.tile([P, SC], F32, tag=f"a_ps{g}")
                nc.tensor.matmul(a_ps, bkv[g], qf_sb[:, g, csl],
                                 start=True, stop=True)
                rden = sb_b.tile([P, SC], F32, tag=f"rden{g}")
                nc.vector.reciprocal_approx_fast(out=rden, in_=d_ps)
                a_sb = sb_b.tile([P, SC], BF16, tag=f"att{g}")
                nc.vector.tensor_tensor(a_sb, a_ps, rden, ALU.mult)
                att.append(a_sb)

            for et in range(NET):
                esl = slice(et * P, (et + 1) * P)
                o_ps = ps_o.tile([P, SC], F32, tag="o_ps")
                for g in range(NG):
                    nc.tensor.matmul(o_ps, wo_sb[:, g, esl], att[g],
                                     start=(g == 0), stop=(g == NG - 1))
                o_sb = sb_o.tile([P, SC], BF16, tag="o_sb")
                if use_biases:
                    if et % 2 == 0:
                        nc.vector.tensor_scalar(
                            o_sb, o_ps, bo_sb[:, et:et + 1], None, op0=ALU.add)
                    else:
                        nc.scalar.activation(
                            o_sb, o_ps, AF.Identity, bias=bo_sb[:, et:et + 1])
                else:
                    if et % 2 == 0:
                        nc.vector.tensor_copy(out=o_sb, in_=o_ps)
                    else:
                        nc.scalar.copy(out=o_sb, in_=o_ps)
                nc.sync.dma_start(out=outT[:, et, csl], in_=o_sb)


def build_nc(use_biases):
    nc = bacc.Bacc("TRN2", target_bir_lowering=False, debug=False)
    t = {}
    t["xT"] = nc.dram_tensor("xT", [P, D * S // P], BF16, kind="ExternalInput")
    t["wqT"] = nc.dram_tensor("wqT", [P, D * CSL // P], BF16, kind="ExternalInput")
    t["wkvT"] = nc.dram_tensor(
        "wkvT", [P, D * 2 * CSL // P], BF16, kind="ExternalInput")
    t["woT"] = nc.dram_tensor("woT", [P, CSL * D // P], BF16, kind="ExternalInput")
    if use_biases:
        t["bq2"] = nc.dram_tensor("bq2", [P, NG], F32, kind="ExternalInput")
        t["bk1"] = nc.dram_tensor("bk1", [CSL], F32, kind="ExternalInput")
        t["bv1"] = nc.dram_tensor("bv1", [CSL], F32, kind="ExternalInput")
        t["bo8"] = nc.dram_tensor("bo8", [P, NET], F32, kind="ExternalInput")
    t["outT"] = nc.dram_tensor("outT", [D, S], BF16, kind="ExternalOutput")

    with tile.TileContext(nc) as tc:
        with ExitStack() as ctx:
            _build_kernel_body(ctx, tc, t, use_biases)
    nc.compile()
    return nc


def _get_nc(use_biases):
    key = ("nc", use_biases)
    if key not in _CACHE:
        _CACHE[key] = build_nc(use_biases)
    return _CACHE[key]


def make_in_maps(x, wq, bq, wk, bk, wv, bv, wo, bo, use_biases=None):
    """Shard the full inputs into the 8 per-core input maps."""
    f = lambda a: np.asarray(a, dtype=np.float32)
    x, wq, bq, wk, bk = f(x), f(wq), f(bq), f(wk), f(bk)
    wv, bv, wo, bo = f(wv), f(bv), f(wo), f(bo)
    bf = lambda a: np.ascontiguousarray(a).astype(ml_dtypes.bfloat16)
    if use_biases is None:
        use_biases = any(np.any(b) for b in (bq, bk, bv, bo))

    # Pre-swizzle to per-partition-contiguous layouts: dram row p holds all
    # of partition p's data back-to-back (d-tile index moved inside).
    def swz(a):  # [(do p), f] -> [p, (do f)]
        dd, f = a.shape
        return bf(a.reshape(dd // P, P, f).transpose(1, 0, 2).reshape(P, -1))

    def swz_x(a):  # [(do p), (h s)] -> [p, (h do s)]
        return bf(a.reshape(NDT, P, 2 * NSC, SC // 2)
                  .transpose(1, 2, 0, 3).reshape(P, -1))

    in_maps = []
    for cid in range(8):
        b, hg = divmod(cid, 4)
        hs = slice(hg * CSL, (hg + 1) * CSL)
        m = {
            "xT": swz_x(x[b].T),
            "wqT": swz(wq[hs, :].T),
            "wkvT": swz(np.concatenate([wk[hs, :].T, wv[hs, :].T], axis=1)),
            "woT": swz(wo[:, hs].T),
        }
        if use_biases:
            bo_in = bo if hg == 0 else np.zeros_like(bo)
            m["bq2"] = np.ascontiguousarray(bq[hs].reshape(NG, P).T)
            m["bk1"] = bk[hs].copy()
            m["bv1"] = bv[hs].copy()
            m["bo8"] = np.ascontiguousarray(bo_in.reshape(NET, P).T)
        in_maps.append(m)
    return in_maps, use_biases


def unshard(results):
    """Sum head-group partials per batch and undo the output transpose."""
    out = np.zeros((BATCH, S, D), np.float32)
    for cid in range(8):
        b = cid // 4
        out[b] += results[cid]["outT"].T.astype(np.float32)
    return out


def kernel(x, wq, bq, wk, bk, wv, bv, wo, bo):
    in_maps, use_biases = make_in_maps(x, wq, bq, wk, bk, wv, bv, wo, bo)
    nc = _get_nc(use_biases)
    res = bass_utils.run_bass_kernel_spmd(nc, in_maps, core_ids=list(range(8)))
    return unshard(res.results)
